# revision 34
# baseline (speedup 1.0000x reference)
"""Distributed attention kernel for 8 TRN2 NeuronCores.

Sharding: core c -> (batch b = c // 4, head-group g = c % 4).
Each core computes, for its batch element, 4 of the 16 heads end-to-end
(QKV projection, rotary, attention, output projection), producing a
partial output for the full [S, D] result. The host sums the 4 group
partials per batch element (the "all-reduce after wo" done at unshard).

Schedule: the attention softmax exp runs on the ACT engine at ~612ns
per [128,512] tile while the PE needs only ~426ns of matmul per tile,
so a straight phase-ordered kernel is ACT-bound during attention
(sim: 157us window for 112us of PE work). This version interleaves
independent projection / output matmuls into the attention instruction
stream ("zipping") so the PE never waits on the ACT:

  1. qk projection for head 0 only (ec-pair (q0,k0), st-inner)
  2. v projection (all heads)
  3. for h in 0..3: attention blocks (it=0..3, h), with the qk
     projection of head h+1 and (for h=3) the wo output projection
     pulled into the stream one matmul at a time by a deficit counter
     that tracks the ACT-vs-PE time imbalance.

Engine placement per zipped block window (~13.6us of PE work):
  ACT : 16 exps (9.8us) + partition-half swap copies (2.6us)
  DVE : rotary muls/sub (2.6us) + f32r denominator accumulation +
        reciprocal + final division (6us)
  Pool: bf16 pair-adds of exp tiles for the denominator (6.4us)

Layouts (host-prepped so the device does zero transposes):
  - xT    [D, S]   : x[b].T
  - wqk   [D, 1024]: per head h: [q_h | k_h] rotary-pair-permuted rows,
                     transposed; cols h*256..h*256+128 = q_h (scaled by
                     1/sqrt(hd)), +128..+256 = k_h
  - wv    [D, 512] : v weight rows transposed
  - wo    [512, D] : wo columns for this group, transposed
  - tabc  [128, S] : cos table doubled across both partition halves
  - tabs  [128, S] : [sin; -sin] so rotary is one mul + one mul + one
                     full-tile subtract (sign folded into the table)

Rotary trick: q/k weight rows are permuted per head so dims [0:64] are
the even (real) rotary components and [64:128] the odd (imag) ones.
Scores are invariant to this permutation since q and k share it.

Attention is computed transposed (scores^T[j, i]) so the softmax
numerator AND attn@v need no transposes. The softmax denominator is an
elementwise sum of the exp tiles (bf16 pair adds on Pool, f32r combines
on DVE) finished by one float32r ones-matmul per i-tile (cross-partition
reduce whose psum rows all equal l -- a free partition broadcast); the
division is applied to the raw attn@v output.
"""

import numpy as np
import ml_dtypes

import concourse.tile as tile
from concourse import bacc, mybir
from concourse.bass_utils import run_bass_kernel_spmd

B, S, D = 2, 2048, 2048
NH, HD = 16, 128
N_CORES = 8
GROUPS = 4
LH = NH // GROUPS  # 4 local heads
EQK = 2 * LH * HD  # 1024 (per-head q|k chunks)
EV = LH * HD  # 512
P = 128
DC = D // P  # 16 contraction chunks over d
SC = S // P  # 16 chunks over s
F = 512  # matmul moving free dim (1 PSUM bank of f32)
NT = S // F  # 4

CDT = mybir.dt.bfloat16
NP_CDT = ml_dtypes.bfloat16
F32 = mybir.dt.float32
F32R = mybir.dt.float32r
NP_OUT = NP_CDT  # device out dtype (partials; host upcasts + sums)

# per-instruction PE/ACT time model used only to pace the zipper
MM_NS = 213  # 512-row bf16 matmul
CHUNK_DEFICIT = 244  # ACT exp (~670ns incl overhead) - 2 matmuls (426ns)

MM_LABELS = []  # emission-order matmul labels (profiling aid; reset per build)


def build_graph(num_devices: int = N_CORES, reps: int = 1):
    """reps > 1 replicates the whole computation (timing instrumentation)."""
    nc = bacc.Bacc(
        "TRN2", target_bir_lowering=False, debug=False, num_devices=num_devices
    )
    xT = nc.dram_tensor("xT", [D, S], CDT, kind="ExternalInput").ap()
    wqk = nc.dram_tensor("wqk", [D, EQK], CDT, kind="ExternalInput").ap()
    wv = nc.dram_tensor("wv", [D, EV], CDT, kind="ExternalInput").ap()
    wo = nc.dram_tensor("wo", [EV, D], CDT, kind="ExternalInput").ap()
    tabc = nc.dram_tensor("tabc", [P, S], CDT, kind="ExternalInput").ap()
    tabs = nc.dram_tensor("tabs", [P, S], CDT, kind="ExternalInput").ap()
    out = nc.dram_tensor("out", [S, D], CDT, kind="ExternalOutput").ap()

    xT_r = xT.rearrange("(c p) s -> p c s", p=P)  # [128, 16, 2048]
    wqk_r = wqk.rearrange("(c p) e -> p c e", p=P)  # [128, 16, 1024]
    wv_r = wv.rearrange("(c p) e -> p c e", p=P)  # [128, 16, 512]
    wo_r = wo.rearrange("(c p) o -> p c o", p=P)  # [128, 4, 2048]
    out_r = out.rearrange("(c p) o -> c p o", p=P)  # [16, 128, 2048]

    Exp = mybir.ActivationFunctionType.Exp
    sub = mybir.AluOpType.subtract

    with tile.TileContext(nc) as tc:
        with (
            tc.tile_pool(name="big", bufs=1) as big,  # xT slot
            tc.tile_pool(name="wqkp", bufs=1) as wqkp,
            tc.tile_pool(name="wvp", bufs=1) as wvp,  # wv slot, reused for wo
            tc.tile_pool(name="data", bufs=1) as data,
            tc.tile_pool(name="tmp", bufs=2) as tmpp,  # qsw,t2
            tc.tile_pool(name="expp", bufs=7) as expp,
            tc.tile_pool(name="small", bufs=1) as small,  # rl
            tc.tile_pool(name="acclp", bufs=5) as acclp,  # accl x4 streams
            tc.tile_pool(name="ostage", bufs=5) as ostagep,
            tc.tile_pool(name="pop", bufs=4, space="PSUM") as pop,
            tc.tile_pool(name="genp", bufs=4, space="PSUM") as genp,
        ):
          for _rep in range(reps):
            # ---------------- loads ----------------
            # Ordered so head 0's qk projection can start ASAP: wqk head-0
            # cols + x st0 + tab st0 first, then x st1-3 / wv / remaining
            # wqk interleaved roughly in consumption order.
            x_sb = big.tile([P, DC, S], CDT, tag="big")
            wqk_sb = wqkp.tile([P, DC, EQK], CDT, tag="wqk")
            wv_sb = wvp.tile([P, DC, EV], CDT, tag="wv")
            tabc_sb = data.tile([P, S], CDT, tag="tabc")
            tabs_sb = data.tile([P, S], CDT, tag="tabs")

            # Single hwdge queue is FIFO at ~210GB/s; order by (a) WAR release
            # time in the PREVIOUS rep (x/wqk/tabs are last read ~70% into a
            # rep, wv slot is reused for wo so it frees last) so back-to-back
            # reps pipeline, and (b) first-rep latency (head-0 qk needs wqk
            # head-0 cols + x st0 first).
            for c in range(DC):
                nc.sync.dma_start(wqk_sb[:, c, 0:256], wqk_r[:, c, 0:256])
            for c in range(DC):
                nc.sync.dma_start(x_sb[:, c, 0:F], xT_r[:, c, 0:F])
            nc.sync.dma_start(tabc_sb[:, 0:F], tabc[:, 0:F])
            nc.sync.dma_start(tabs_sb[:, 0:F], tabs[:, 0:F])
            for c in range(DC):
                nc.sync.dma_start(wqk_sb[:, c, 256:EQK], wqk_r[:, c, 256:EQK])
            for st in range(1, NT):
                for c in range(DC):
                    nc.sync.dma_start(
                        x_sb[:, c, st * F : (st + 1) * F],
                        xT_r[:, c, st * F : (st + 1) * F],
                    )
                nc.sync.dma_start(
                    tabc_sb[:, st * F : (st + 1) * F], tabc[:, st * F : (st + 1) * F]
                )
                nc.sync.dma_start(
                    tabs_sb[:, st * F : (st + 1) * F], tabs[:, st * F : (st + 1) * F]
                )
            # wv last among inputs: its slot (reused for wo) frees only at
            # the END of the previous rep, and a waiting DMA blocks its
            # issuing SEQ past the 4-deep park -- so it must sit behind the
            # x/wqk prefetch, and NOT on the ACT queue (it would block exps).
            for c in range(DC):
                nc.sync.dma_start(wv_sb[:, c, :], wv_r[:, c, :])

            rot_sb = data.tile([P, 2 * LH, S], CDT, tag="rot")
            v_sb = data.tile([P, SC, EV], CDT, tag="v")
            attn_sb = data.tile([P, LH, S], CDT, tag="attn")
            ones_bf = data.tile([P, P], CDT, tag="ones")
            nc.vector.memset(ones_bf[:], 1.0)

            # -------- emission helpers --------
            def rot_consumers(ps, ec, st):
                """Rotary on a finished qk psum group -> rot_sb[:, ec, sl].
                ACT does the partition-half swap copy; DVE the 3 tensor ops.
                partitions 0:64 = even (re), 64:128 = odd (im); tabs holds
                [sin; -sin] so one full-tile subtract finishes both halves."""
                sl = slice(st * F, (st + 1) * F)
                qsw = tmpp.tile([P, F], CDT, tag="qsw")
                nc.scalar.copy(out=qsw[0:64], in_=ps[64:128])
                nc.scalar.copy(out=qsw[64:128], in_=ps[0:64])
                t2 = tmpp.tile([P, F], CDT, tag="t2")
                nc.vector.tensor_mul(rot_sb[:, ec, sl], ps[:], tabc_sb[:, sl])
                nc.vector.tensor_mul(t2[:], qsw[:], tabs_sb[:, sl])
                nc.vector.tensor_tensor(
                    rot_sb[:, ec, sl], rot_sb[:, ec, sl], t2[:], sub
                )

            def gen_wo_set(it):
                """Yield once per matmul of the wo groups for sc in this
                it-set; psum->bf16 staging on ACT; DMA out."""
                for sc in range(4 * it, 4 * it + 4):
                    for ot in range(NT):
                        osl = slice(ot * F, (ot + 1) * F)
                        pw = genp.tile([P, F], F32, tag="gen")
                        for hc in range(LH):
                            MM_LABELS.append(f"wo{it}.sc{sc}.ot{ot}.hc{hc}")
                            nc.tensor.matmul(
                                pw[:],
                                lhsT=attn_sb[:, hc, sc * P : (sc + 1) * P],
                                rhs=wo_sb[:, hc, osl],
                                start=(hc == 0),
                                stop=(hc == LH - 1),
                            )
                            yield MM_NS
                        ost = ostagep.tile([P, F], CDT, tag="ostage")
                        # DVE, not ACT: the ACT engine is ~93% busy with exps
                        # during the zip phase and ostage copies there starve
                        # the attnv stream
                        nc.vector.tensor_copy(out=ost[:], in_=pw[:])
                        nc.sync.dma_start(out_r[sc, :, osl], ost[:])

            # zipper state: queue of filler generators + time-deficit counter
            filler_q = []
            zstate = {"deficit": 0.0}

            def pull_filler(ns):
                zstate["deficit"] += ns
                if not filler_q:
                    zstate["deficit"] = min(zstate["deficit"], 4 * MM_NS)
                while filler_q and zstate["deficit"] >= MM_NS:
                    try:
                        cost = next(filler_q[0])
                        zstate["deficit"] -= cost
                    except StopIteration:
                        filler_q.pop(0)

            def drain_fillers(n_keep=0):
                while len(filler_q) > n_keep:
                    try:
                        next(filler_q[0])
                    except StopIteration:
                        filler_q.pop(0)

            def drain_one_vproj():
                # head of the queue is the vproj generator while it lives
                try:
                    next(filler_q[0])
                except StopIteration:
                    filler_q.pop(0)

            # ---------------- qk projection (all heads) ----------------
            # st-outer so the first groups only need x st0; frees x/wqk by
            # ~40% of the rep so the next rep's DMA prefetch has room.
            # q_h cols = h*256..+128 -> rot slot h; k_h = +128..+256 -> LH+h.
            for st in range(NT):
                for h in range(LH):
                    for qk in range(2):
                        col0 = h * 256 + qk * P
                        dst = h + qk * LH
                        ps = genp.tile([P, F], F32, tag="gen")
                        for c in range(DC):
                            MM_LABELS.append(f"qk{h}.{qk}.st{st}.c{c}")
                            nc.tensor.matmul(
                                ps[:],
                                lhsT=wqk_sb[:, c, col0 : col0 + P],
                                rhs=x_sb[:, c, st * F : (st + 1) * F],
                                start=(c == 0),
                                stop=(c == DC - 1),
                            )
                        rot_consumers(ps, dst, st)

            # ---------------- v projection ----------------
            # v[s, e] = sum_d xT[d, s] * wv[d, e]; psum -> v_sb copy on ACT.
            # Only sc0-1 are computed up front; the rest zip just-in-time
            # into the first attention it-set (chunk-major over h makes one
            # v chunk feed 4 attnv chunks, so the deadline is loose).
            vstate = {"done": 0}

            def emit_vproj(sc):
                ps = genp.tile([P, F], F32, tag="gen")
                for c in range(DC):
                    MM_LABELS.append(f"v.sc{sc}.c{c}")
                    nc.tensor.matmul(
                        ps[:],
                        lhsT=x_sb[:, c, sc * P : (sc + 1) * P],
                        rhs=wv_sb[:, c, :],
                        start=(c == 0),
                        stop=(c == DC - 1),
                    )
                nc.scalar.copy(out=v_sb[:, sc, :], in_=ps[:])
                vstate["done"] = sc + 1

            def gen_vproj():
                for sc in range(2, SC):
                    ps = genp.tile([P, F], F32, tag="gen")
                    for c in range(DC):
                        MM_LABELS.append(f"v.sc{sc}.c{c}")
                        nc.tensor.matmul(
                            ps[:],
                            lhsT=x_sb[:, c, sc * P : (sc + 1) * P],
                            rhs=wv_sb[:, c, :],
                            start=(c == 0),
                            stop=(c == DC - 1),
                        )
                        yield MM_NS
                    nc.scalar.copy(out=v_sb[:, sc, :], in_=ps[:])
                    vstate["done"] = sc + 1

            for sc in range(2):
                emit_vproj(sc)

            # wo weights into the (now dead) wv slot
            wo_sb = wvp.tile([P, LH, D], CDT, tag="wv")
            for c in range(LH):
                nc.sync.dma_start(wo_sb[:, c, :], wo_r[:, c, :])

            # ---------------- attention, zipped, chunk-major ----------------
            # Within each it-set the 4 head-streams interleave chunk-major
            # ((jc, h) order) so one v chunk unlocks 4 attnv chunks: the
            # remaining v projection zips just-in-time into it-set 0, and
            # wo sets (unlocked per it-set) cover the rest. 4 po banks hold
            # the 4 concurrent streams; scores/fillers share the other 4.
            LAG = 2
            state = {}  # (it, h) -> dict(po, accl, ets)

            def chunk_score(it, h, j):
                if j == 0:
                    state[(it, h)] = {
                        "po": pop.tile([P, F], F32, tag="po", name=f"po{it}_{h}"),
                        "accl": acclp.tile(
                            [P, F], CDT, tag="accl", name=f"accl{it}_{h}"
                        ),
                        "ets": {},
                    }
                ps = genp.tile([P, F], F32, tag="gen")
                # scores^T[j, i] = sum_hd k[hd, j] * q[hd, i]
                MM_LABELS.append(f"s.{it}.{h}.j{j}")
                nc.tensor.matmul(
                    ps[:],
                    lhsT=rot_sb[:, LH + h, j * P : (j + 1) * P],
                    rhs=rot_sb[:, h, it * F : (it + 1) * F],
                    start=True,
                    stop=True,
                )
                pull_filler(CHUNK_DEFICIT)
                et = expp.tile([P, F], CDT, tag="exp")
                nc.scalar.activation(out=et[:], in_=ps[:], func=Exp)
                state[(it, h)]["ets"][j] = et

            def chunk_attnv(it, h, jj):
                st_ = state[(it, h)]
                et = st_["ets"].pop(jj)
                MM_LABELS.append(f"a.{it}.{h}.j{jj}")
                nc.tensor.matmul(
                    st_["po"][:],
                    lhsT=v_sb[:, jj, h * P : (h + 1) * P],
                    rhs=et[:],
                    start=(jj == 0),
                    stop=(jj == SC - 1),
                )
                pull_filler(0)
                # denominator: direct bf16 accumulate on DVE (2x mode)
                if jj == 0:
                    nc.vector.tensor_copy(out=st_["accl"][:], in_=et[:])
                else:
                    nc.vector.tensor_add(st_["accl"][:], st_["accl"][:], et[:])
                if jj == SC - 1:
                    pending_fin.append((it, h))

            def finish_stream(it, h):
                st_ = state[(it, h)]
                pl = genp.tile([P, F], F32, tag="gen")
                MM_LABELS.append(f"l.{it}.{h}")
                nc.tensor.matmul(
                    pl[:], lhsT=ones_bf[:], rhs=st_["accl"][:], start=True, stop=True
                )
                pull_filler(0)
                # pl rows are all equal (ones lhsT) -> reciprocal is already
                # "broadcast" across partitions.
                rl = small.tile([P, F], F32, tag="recip")
                nc.vector.reciprocal_approx_fast(rl[:], pl[:])
                nc.vector.tensor_mul(
                    attn_sb[:, h, it * F : (it + 1) * F], st_["po"][:], rl[:]
                )
                del state[(it, h)]
                if h == LH - 1:
                    filler_q.append(gen_wo_set(it))

            vgen_pushed = False
            pending_fin = []
            for it in range(NT):
                if not vgen_pushed:
                    filler_q.insert(0, gen_vproj())
                    vgen_pushed = True
                chunks = [(jc, h) for jc in range(SC) for h in range(LH)]
                NCH = len(chunks)
                for p in range(NCH + LAG):
                    if p < NCH:
                        jc, h = chunks[p]
                        chunk_score(it, h, jc)
                    # deferred stream finishes: paced into positions 4+
                    # of the NEXT it-set so the lsum's wait on the accl
                    # add-chain (~3us of DVE queue) fully hides under the
                    # next set's scores instead of bunching at the boundary
                    if pending_fin and p >= 4 and p % 2 == 0:
                        finish_stream(*pending_fin.pop(0))
                    pp = p - LAG
                    if pp >= 0:
                        jc, h = chunks[pp]
                        # deadline: v chunk jc must exist before any attnv
                        # that reads it (only binds during it-set 0)
                        while vstate["done"] <= jc:
                            drain_one_vproj()
                        chunk_attnv(it, h, jc)

            drain_fillers()
            while pending_fin:
                finish_stream(*pending_fin.pop(0))
                # space the final lsums with wo filler matmuls so the
                # in-order PE isn't head-blocked on 4 accl chains in a row
                pull_filler(2000)
            drain_fillers()

    nc.compile()
    return nc


def shard_inputs(x, freqs_cis, wqkv, wo):
    """Produce the 8 per-core input maps (host-side layout prep)."""
    x = np.asarray(x, dtype=np.float32)
    freqs_cis = np.asarray(freqs_cis, dtype=np.float32)
    wqkv = np.asarray(wqkv, dtype=np.float32)
    wo = np.asarray(wo, dtype=np.float32)

    perm = np.concatenate([np.arange(0, HD, 2), np.arange(1, HD, 2)])  # even|odd
    cos = freqs_cis[:, :, 0].T  # [64, S]
    sin = freqs_cis[:, :, 1].T
    scale = 1.0 / np.sqrt(HD)  # folded into wq rows below
    tabc = np.concatenate([cos, cos], axis=0)  # [128, S]
    tabs = np.concatenate([sin, -sin], axis=0)  # sign folded for one-sub rotary

    tabc = np.ascontiguousarray(tabc.astype(NP_CDT))
    tabs = np.ascontiguousarray(tabs.astype(NP_CDT))

    in_maps = []
    for c in range(N_CORES):
        b, g = divmod(c, GROUPS)
        heads = range(g * LH, (g + 1) * LH)
        # per head: [q_h (scaled, perm) | k_h (perm)] -> wqk cols h*256..
        blocks = []
        for h in heads:
            blocks.append(wqkv[h * HD : (h + 1) * HD][perm] * scale)
            blocks.append(wqkv[D + h * HD : D + (h + 1) * HD][perm])
        wqk_l = np.concatenate(blocks, axis=0).T  # [D, 1024]
        wv_rows = np.concatenate(
            [wqkv[2 * D + h * HD : 2 * D + (h + 1) * HD] for h in heads], axis=0
        )
        wv_l = wv_rows.T  # [D, 512]
        din = np.concatenate([np.arange(h * HD, (h + 1) * HD) for h in heads])
        wo_l = wo[:, din].T  # [512, D]
        in_maps.append(
            {
                "xT": np.ascontiguousarray(x[b].T.astype(NP_CDT)),
                "wqk": np.ascontiguousarray(wqk_l.astype(NP_CDT)),
                "wv": np.ascontiguousarray(wv_l.astype(NP_CDT)),
                "wo": np.ascontiguousarray(wo_l.astype(NP_CDT)),
                "tabc": tabc,
                "tabs": tabs,
            }
        )
    return in_maps


def unshard_outputs(results):
    out = np.zeros((B, S, D), dtype=np.float32)
    for c in range(N_CORES):
        b = c // GROUPS
        out[b] += results[c]["out"].astype(np.float32)
    return out


_GRAPH_CACHE = {}


def kernel(x, freqs_cis, wqkv, wo):
    if "nc" not in _GRAPH_CACHE:
        _GRAPH_CACHE["nc"] = build_graph()
    nc = _GRAPH_CACHE["nc"]
    in_maps = shard_inputs(x, freqs_cis, wqkv, wo)
    res = run_bass_kernel_spmd(nc, in_maps, core_ids=list(range(N_CORES)))
    return unshard_outputs(res.results)
